# revision 7
# baseline (speedup 1.0000x reference)
"""Trainium2 Bass kernel for the spiking CNN (nn_CNNModel_47785806135777).

Key facts exploited:
  - The reference's straight-through graph is numerically identity in the
    forward pass: output == (Tf0 @ wf1.T) / (VTH * STEPS), where Tf0 is the
    fc-layer spike count.  Only the membranes + Tf0 must be computed.
  - The Poisson randomness is jax.random.key(42) threefry — bit-exactly
    reproducible on host CPU; spikes are precomputed and streamed in.
  - All matmul moving operands are binary spikes (exact in bf16); fp32
    weights are split into an exact 3-term bf16 sum, so every matmul runs
    at full bf16 PE speed with fp32-class accuracy.
  - avgpool is a linear map -> done on the tensor engine as a 0.25-valued
    matmul (exact in fp32 PSUM since spikes are 0/1).
Data parallel across 8 cores: 64 images per core.
"""

import numpy as np
import ml_dtypes

STEPS = 100
VTH = 1.0
POOL_TH = 0.75
B = 512
NCORES = 8
BL = B // NCORES  # 64

BF16 = ml_dtypes.bfloat16

_BUILD_CACHE = {}


# ---------------------------------------------------------------- host prep

def _spikes_all(inputdata):
    import jax
    import jax.numpy as jnp

    cpu = jax.devices("cpu")[0]
    with jax.default_device(cpu):
        keys = jax.random.split(jax.random.key(42), STEPS)
        x = jnp.asarray(inputdata)
        half = jnp.abs(x) / 2.0
        sgn = jnp.sign(x)

        @jax.jit
        def gen(keys):
            def body(_, k):
                r = jax.random.uniform(k, x.shape, dtype=jnp.float32)
                return None, (half > r).astype(jnp.float32) * sgn
            _, s = jax.lax.scan(body, None, keys)
            return s

        return np.asarray(gen(keys))  # (STEPS, B, 1, 28, 28)


def _split3(w):
    terms = []
    r = np.asarray(w, dtype=np.float32).copy()
    for _ in range(3):
        t = r.astype(BF16)
        terms.append(t)
        r = r - t.astype(np.float32)
    return terms


def _conv1_lhsT(w1s, ypack):
    # [50, 20*ypack]; k = dyp*5 + dx ; m = co*ypack + yp
    L = np.zeros((50, 20 * ypack), dtype=BF16)
    w = w1s  # bf16 (20,1,5,5)
    for dyp in range(10):
        for dx in range(5):
            k = dyp * 5 + dx
            for yp in range(ypack):
                dy = dyp - yp
                if 0 <= dy < 5:
                    for co in range(20):
                        L[k, co * ypack + yp] = w[co, 0, dy, dx]
    return L


def _conv2_lhsT(w2s, dx):
    # [120, 100]; k = dyp*20 + ci ; m = co*2 + yp
    L = np.zeros((120, 100), dtype=BF16)
    for dyp in range(6):
        for ci in range(20):
            k = dyp * 20 + ci
            for yp in range(2):
                dy = dyp - yp
                if 0 <= dy < 5:
                    for co in range(50):
                        L[k, co * 2 + yp] = w2s[co, ci, dy, dx]
    return L


def _pool_lhsT(nch, ypack):
    # [nch*ypack, nch*(ypack//2)] : 0.25 where co==co' and yp//2==y'p
    yo = ypack // 2
    L = np.zeros((nch * ypack, nch * yo), dtype=BF16)
    for co in range(nch):
        for yp in range(ypack):
            L[co * ypack + yp, co * yo + yp // 2] = 0.25
    return L


def _host_tensors(w1, w2, wf0, wf1):
    w1s = _split3(w1)
    w2s = _split3(w2)
    wf0s = _split3(wf0)

    # conv1: [3*50, 120] and [3*50, 80]
    c1w = np.concatenate([_conv1_lhsT(s, 6) for s in w1s], axis=0)
    c1wl = np.concatenate([_conv1_lhsT(s, 4) for s in w1s], axis=0)
    # conv2: [15*120, 100]  (s-major, then dx)
    c2w = np.concatenate(
        [_conv2_lhsT(w2s[s], dx) for s in range(3) for dx in range(5)], axis=0
    )
    p1w = _pool_lhsT(20, 6)      # [120, 60]
    p1wl = _pool_lhsT(20, 4)     # [80, 40]
    p2w = _pool_lhsT(50, 2)      # [100, 50]
    # fc: rows f = co*49 + y''*7 + x'' ; [3, 20, 128, 200] flattened
    fcw = np.zeros((3, 2560, 200), dtype=BF16)
    for s in range(3):
        fcw[s, :2450, :] = wf0s[s].T
    fcw = fcw.reshape(3 * 20, 128, 200).reshape(3 * 20 * 128, 200)
    # wf1 packed: [128, 20] fp32
    wf1p = np.zeros((128, 20), dtype=np.float32)
    wf1T = np.asarray(wf1, np.float32).T  # [200, 10]
    wf1p[:, 0:10] = wf1T[0:128]
    wf1p[:72, 10:20] = wf1T[128:200]
    return dict(c1w=c1w, c1wl=c1wl, c2w=c2w, p1w=p1w, p1wl=p1wl, p2w=p2w,
                fcw=fcw, wf1p=wf1p)


# ---------------------------------------------------------------- bass build

def _build(steps):
    import concourse.bass as bass
    import concourse.mybir as mybir
    import concourse.tile as tile
    from concourse import bacc

    dt = mybir.dt
    Alu = mybir.AluOpType

    nc = bacc.Bacc(trn_type="TRN2")

    spikes_h = nc.dram_tensor("spikes", [steps, 69632], dt.bfloat16, kind="ExternalInput")
    c1w_h = nc.dram_tensor("c1w", [150, 120], dt.bfloat16, kind="ExternalInput")
    c1wl_h = nc.dram_tensor("c1wl", [150, 80], dt.bfloat16, kind="ExternalInput")
    c2w_h = nc.dram_tensor("c2w", [1800, 100], dt.bfloat16, kind="ExternalInput")
    p1w_h = nc.dram_tensor("p1w", [120, 60], dt.bfloat16, kind="ExternalInput")
    p1wl_h = nc.dram_tensor("p1wl", [80, 40], dt.bfloat16, kind="ExternalInput")
    p2w_h = nc.dram_tensor("p2w", [100, 50], dt.bfloat16, kind="ExternalInput")
    fcw_h = nc.dram_tensor("fcw", [7680, 200], dt.bfloat16, kind="ExternalInput")
    wf1p_h = nc.dram_tensor("wf1p", [128, 20], dt.float32, kind="ExternalInput")
    out_h = nc.dram_tensor("out", [10, 64], dt.float32, kind="ExternalOutput")

    YB = [(0, 6), (1, 6), (2, 6), (3, 6), (4, 4)]  # (ybl, ypack)

    with tile.TileContext(nc) as tc:
        import contextlib
        ctx = contextlib.ExitStack()
        with ctx:
            state = ctx.enter_context(tc.tile_pool(name="state", bufs=1))
            dram = ctx.enter_context(tc.tile_pool(name="dram", bufs=1, space="DRAM"))
            work = ctx.enter_context(tc.tile_pool(name="work", bufs=2))
            patches_p = ctx.enter_context(tc.tile_pool(name="patches", bufs=3))
            o1_p = ctx.enter_context(tc.tile_pool(name="o1p", bufs=2))
            o3_p = ctx.enter_context(tc.tile_pool(name="o3p", bufs=2))
            ps_c1 = ctx.enter_context(tc.tile_pool(name="psc1", bufs=2, space="PSUM"))
            ps_c2 = ctx.enter_context(tc.tile_pool(name="psc2", bufs=2, space="PSUM"))
            ps_p1 = ctx.enter_context(tc.tile_pool(name="psp1", bufs=1, space="PSUM"))
            ps_p2 = ctx.enter_context(tc.tile_pool(name="psp2", bufs=1, space="PSUM"))
            ps_fc = ctx.enter_context(tc.tile_pool(name="psfc", bufs=1, space="PSUM"))
            ps_fc2 = ctx.enter_context(tc.tile_pool(name="psfc2", bufs=1, space="PSUM"))

            # ---- persistent state
            m1 = [state.tile([20 * yp, 1792], dt.float32, tag=f"m1_{i}", name=f"m1_{i}") for i, yp in YB]
            m1s = [state.tile([10 * yp, 896], dt.float32, tag=f"m1s_{i}", name=f"m1s_{i}") for i, yp in YB]
            m2 = state.tile([100, 6272], dt.float32, tag="m2", name="m2")
            m2s = state.tile([50, 3136], dt.float32, tag="m2s", name="m2s")
            mf0a = state.tile([128, 64], dt.float32, tag="mf0a", name="mf0a")
            mf0b = state.tile([72, 64], dt.float32, tag="mf0b", name="mf0b")
            tf0a = state.tile([128, 64], dt.float32, tag="tf0a", name="tf0a")
            tf0b = state.tile([72, 64], dt.float32, tag="tf0b", name="tf0b")
            zcol = state.tile([128, 1], dt.float32, tag="zcol", name="zcol")

            # ---- weights in SBUF
            c1w_sb = state.tile([50, 360], dt.bfloat16, tag="c1w", name="c1w")
            c1wl_sb = state.tile([50, 240], dt.bfloat16, tag="c1wl", name="c1wl")
            c2w_sb = state.tile([120, 1500], dt.bfloat16, tag="c2w", name="c2w")
            p1w_sb = state.tile([120, 60], dt.bfloat16, tag="p1w", name="p1w")
            p1wl_sb = state.tile([80, 40], dt.bfloat16, tag="p1wl", name="p1wl")
            p2w_sb = state.tile([100, 50], dt.bfloat16, tag="p2w", name="p2w")
            fcw_sb = state.tile([128, 12000], dt.bfloat16, tag="fcw", name="fcw")
            wf1p_sb = state.tile([128, 20], dt.float32, tag="wf1p", name="wf1p")

            # weight DMAs (reshape DRAM rows into sbuf free dims)
            for s in range(3):
                nc.sync.dma_start(out=c1w_sb[:, s * 120:(s + 1) * 120],
                                  in_=c1w_h[s * 50:(s + 1) * 50, :])
                nc.sync.dma_start(out=c1wl_sb[:, s * 80:(s + 1) * 80],
                                  in_=c1wl_h[s * 50:(s + 1) * 50, :])
            for j in range(15):
                nc.sync.dma_start(out=c2w_sb[:, j * 100:(j + 1) * 100],
                                  in_=c2w_h[j * 120:(j + 1) * 120, :])
            nc.sync.dma_start(out=p1w_sb[:], in_=p1w_h[:])
            nc.sync.dma_start(out=p1wl_sb[:], in_=p1wl_h[:])
            nc.sync.dma_start(out=p2w_sb[:], in_=p2w_h[:])
            for j in range(60):
                nc.sync.dma_start(out=fcw_sb[:, j * 200:(j + 1) * 200],
                                  in_=fcw_h[j * 128:(j + 1) * 128, :])
            nc.sync.dma_start(out=wf1p_sb[:], in_=wf1p_h[:])

            # ---- DRAM scratch
            o2_dram = dram.tile([20 * 18 * 18 * 64], dt.bfloat16, tag="o2d", name="o2d")
            o4_dram = dram.tile([2560 * 64], dt.bfloat16, tag="o4d", name="o4d")

            # ---- zero init
            for t in m1 + m1s + [m2, m2s, mf0a, mf0b, tf0a, tf0b, zcol]:
                nc.vector.memset(t[:], 0.0)
            zb = work.tile([128, 3240], dt.bfloat16, tag="zb", name="zb")
            nc.gpsimd.memset(zb[:], 0.0)
            nc.sync.dma_start(
                out=o2_dram.rearrange("(p f) -> p f", p=128), in_=zb[:])
            nc.sync.dma_start(
                out=o4_dram.rearrange("(p f) -> p f", p=128), in_=zb[:, :1280])

            def step_body(iv):
                # ---------------- conv1 + LIF1 + pool1 + LIFp1
                o2s_list = []
                for ybl, ypk in YB:
                    M = 20 * ypk
                    patches = patches_p.tile([50, 1792], dt.bfloat16, tag="patches", name="patches")
                    src = bass.AP(
                        tensor=spikes_h,
                        offset=iv * 69632 + 6 * ybl * 2048,
                        ap=[[2048, 10], [64, 5], [64, 28], [1, 64]],
                    )
                    nc.sync.dma_start(out=patches[:], in_=src)
                    lw = c1w_sb if ypk == 6 else c1wl_sb
                    o1 = o1_p.tile([M, 1792], dt.bfloat16, tag="o1", name="o1")
                    for c in range(4):
                        ps = ps_c1.tile([M, 448], dt.float32, tag="psc1", name="psc1")
                        for s in range(3):
                            nc.tensor.matmul(
                                out=ps[:],
                                lhsT=lw[:, s * M:(s + 1) * M],
                                rhs=patches[:, c * 448:(c + 1) * 448],
                                start=(s == 0), stop=(s == 2),
                            )
                        mm = m1[ybl][:, c * 448:(c + 1) * 448]
                        nc.vector.tensor_tensor(out=mm, in0=mm, in1=ps[:], op=Alu.add)
                    # fire layer1
                    nc.gpsimd.tensor_scalar(out=o1[:], in0=m1[ybl][:], scalar1=VTH,
                                            scalar2=None, op0=Alu.is_gt)
                    nc.vector.copy_predicated(
                        out=m1[ybl][:], mask=o1[:].bitcast(dt.uint16),
                        data=zcol[0:M, 0:1].to_broadcast([M, 1792]))
                    # pool1 (PE): out [10*ypk, 448] x 2 chunks
                    pw = p1w_sb if ypk == 6 else p1wl_sb
                    MP = 10 * ypk
                    o1r = o1[:].rearrange("p (x two b) -> p x two b", two=2, b=64)
                    o2s = work.tile([MP, 896], dt.bfloat16, tag="o2s", name="o2s")
                    for c in range(2):
                        psp = ps_p1.tile([MP, 448], dt.float32, tag="psp1", name="psp1")
                        for par in range(2):
                            nc.tensor.matmul(
                                out=psp[:],
                                lhsT=pw[:],
                                rhs=o1r[:, c * 7:(c + 1) * 7, par, :],
                                start=(par == 0), stop=(par == 1),
                            )
                        mm = m1s[ybl][:, c * 448:(c + 1) * 448]
                        nc.vector.tensor_tensor(out=mm, in0=mm, in1=psp[:], op=Alu.add)
                    nc.vector.tensor_scalar(out=o2s[:], in0=m1s[ybl][:], scalar1=POOL_TH,
                                            scalar2=None, op0=Alu.is_gt)
                    nc.vector.copy_predicated(
                        out=m1s[ybl][:], mask=o2s[:].bitcast(dt.uint16),
                        data=zcol[0:MP, 0:1].to_broadcast([MP, 896]))
                    o2s_list.append((ybl, ypk, o2s))

                # ---------------- o2 -> DRAM (padded [20, 18, 18, 64])
                for ybl, ypk, o2s in o2s_list:
                    yo = ypk // 2
                    dst = bass.AP(
                        tensor=o2_dram.tensor,
                        offset=o2_dram[:].offset + (3 * ybl + 2) * 1152 + 2 * 64,
                        ap=[[20736, 20], [1152, yo], [64, 14], [1, 64]],
                    )
                    srcr = o2s[:].rearrange("(co yo) (x b) -> (co yo) x b", yo=yo, b=64)
                    nc.sync.dma_start(out=dst, in_=srcr)

                # ---------------- conv2 replica + conv2 + LIF2 + pool2 + LIFp2
                rep = work.tile([120, 8064], dt.bfloat16, tag="rep", name="rep")
                for dyp in range(6):
                    src = bass.AP(
                        tensor=o2_dram.tensor,
                        offset=o2_dram[:].offset + dyp * 1152,
                        ap=[[20736, 20], [2304, 7], [64, 18], [1, 64]],
                    )
                    nc.sync.dma_start(out=rep[dyp * 20:(dyp + 1) * 20, :], in_=src)
                repr_ = rep[:].rearrange("p (y x b) -> p y x b", y=7, x=18, b=64)
                o4 = work.tile([50, 3136], dt.bfloat16, tag="o4", name="o4")
                for yb2 in range(7):
                    o3 = o3_p.tile([100, 896], dt.bfloat16, tag="o3", name="o3")
                    for c in range(2):
                        ps = ps_c2.tile([100, 448], dt.float32, tag="psc2", name="psc2")
                        first = True
                        for s in range(3):
                            for dx in range(5):
                                nc.tensor.matmul(
                                    out=ps[:],
                                    lhsT=c2w_sb[:, (s * 5 + dx) * 100:(s * 5 + dx + 1) * 100],
                                    rhs=repr_[:, yb2, dx + c * 7:dx + (c + 1) * 7, :],
                                    start=first, stop=(s == 2 and dx == 4),
                                )
                                first = False
                        mm = m2[:, yb2 * 896 + c * 448: yb2 * 896 + (c + 1) * 448]
                        nc.vector.tensor_tensor(out=mm, in0=mm, in1=ps[:], op=Alu.add)
                    m2c = m2[:, yb2 * 896:(yb2 + 1) * 896]
                    nc.gpsimd.tensor_scalar(out=o3[:], in0=m2c, scalar1=VTH,
                                            scalar2=None, op0=Alu.is_gt)
                    nc.vector.copy_predicated(
                        out=m2c, mask=o3[:].bitcast(dt.uint16),
                        data=zcol[0:100, 0:1].to_broadcast([100, 896]))
                    # pool2
                    o3r = o3[:].rearrange("p (x two b) -> p x two b", two=2, b=64)
                    psp = ps_p2.tile([50, 448], dt.float32, tag="psp2", name="psp2")
                    for par in range(2):
                        nc.tensor.matmul(out=psp[:], lhsT=p2w_sb[:],
                                         rhs=o3r[:, :, par, :],
                                         start=(par == 0), stop=(par == 1))
                    mm = m2s[:, yb2 * 448:(yb2 + 1) * 448]
                    nc.vector.tensor_tensor(out=mm, in0=mm, in1=psp[:], op=Alu.add)
                nc.vector.tensor_scalar(out=o4[:], in0=m2s[:], scalar1=POOL_TH,
                                        scalar2=None, op0=Alu.is_gt)
                nc.vector.copy_predicated(
                    out=m2s[:], mask=o4[:].bitcast(dt.uint16),
                    data=zcol[0:50, 0:1].to_broadcast([50, 3136]))

                # ---------------- o4 -> DRAM -> o4T
                dst = bass.AP(tensor=o4_dram.tensor, offset=o4_dram[:].offset,
                              ap=[[3136, 50], [1, 3136]])
                nc.sync.dma_start(out=dst, in_=o4[:])
                o4T = work.tile([128, 1280], dt.bfloat16, tag="o4T", name="o4T")
                src = bass.AP(tensor=o4_dram.tensor, offset=o4_dram[:].offset,
                              ap=[[64, 128], [8192, 20], [1, 64]])
                nc.sync.dma_start(out=o4T[:], in_=src)

                # ---------------- fc + LIF f0 + Tf0
                psa = ps_fc.tile([128, 64], dt.float32, tag="psfc", name="psfc")
                psb = ps_fc2.tile([72, 64], dt.float32, tag="psfc2", name="psfc2")
                for s in range(3):
                    for k in range(20):
                        base = (s * 20 + k) * 200
                        nc.tensor.matmul(out=psa[:],
                                         lhsT=fcw_sb[:, base:base + 128],
                                         rhs=o4T[:, k * 64:(k + 1) * 64],
                                         start=(s == 0 and k == 0),
                                         stop=(s == 2 and k == 19))
                        nc.tensor.matmul(out=psb[:],
                                         lhsT=fcw_sb[:, base + 128:base + 200],
                                         rhs=o4T[:, k * 64:(k + 1) * 64],
                                         start=(s == 0 and k == 0),
                                         stop=(s == 2 and k == 19))
                for mf, psx, tf, P in ((mf0a, psa, tf0a, 128), (mf0b, psb, tf0b, 72)):
                    nc.vector.tensor_tensor(out=mf[:], in0=mf[:], in1=psx[:], op=Alu.add)
                    o5 = work.tile([P, 64], dt.bfloat16, tag=f"o5_{P}", name=f"o5_{P}")
                    nc.vector.tensor_scalar(out=o5[:], in0=mf[:], scalar1=VTH,
                                            scalar2=None, op0=Alu.is_gt)
                    nc.vector.copy_predicated(
                        out=mf[:], mask=o5[:].bitcast(dt.uint16),
                        data=zcol[0:P, 0:1].to_broadcast([P, 64]))
                    nc.vector.tensor_tensor(out=tf[:], in0=tf[:], in1=o5[:], op=Alu.add)

            with tc.For_i(0, steps, 1) as iv:
                step_body(iv)

            # ---------------- final: out = (Tf0 @ wf1.T) / 100  -> [10, 64]
            pso = ps_fc.tile([10, 64], dt.float32, tag="psfc", name="psfc")
            nc.tensor.matmul(out=pso[:], lhsT=wf1p_sb[:, 0:10], rhs=tf0a[:],
                             start=True, stop=False)
            nc.tensor.matmul(out=pso[:], lhsT=wf1p_sb[0:72, 10:20], rhs=tf0b[:],
                             start=False, stop=True)
            out_sb = work.tile([10, 64], dt.float32, tag="outsb", name="outsb")
            nc.vector.tensor_scalar(out=out_sb[:], in0=pso[:],
                                    scalar1=1.0 / (VTH * STEPS), scalar2=None,
                                    op0=Alu.mult)
            nc.sync.dma_start(out=out_h[:], in_=out_sb[:])

    nc.finalize()
    return nc


# ---------------------------------------------------------------- entry

def kernel(inputdata, w1, w2, wf0, wf1, _steps=STEPS, _trace=False):
    from concourse.bass_utils import run_bass_kernel_spmd

    spikes = _spikes_all(inputdata)  # (STEPS, 512, 1, 28, 28) fp32
    # pad into (steps, 32, 32, 64) per core, bf16
    hw = _host_tensors(w1, w2, wf0, wf1)

    if _steps not in _BUILD_CACHE:
        _BUILD_CACHE[_steps] = _build(_steps)
    nc = _BUILD_CACHE[_steps]

    in_maps = []
    for c in range(NCORES):
        sp = np.zeros((_steps, 34, 32, BL), dtype=BF16)
        # spikes[t, b, 0, y, x] -> sp[t, y+2, x+2, b]
        blk = spikes[:_steps, c * BL:(c + 1) * BL, 0]  # (steps, 64, 28, 28)
        sp[:, 2:30, 2:30, :] = np.transpose(blk, (0, 2, 3, 1)).astype(BF16)
        m = dict(hw)
        m["spikes"] = sp.reshape(_steps, 69632)
        in_maps.append(m)

    import time as _time
    _t0 = _time.time()
    res = run_bass_kernel_spmd(nc, in_maps, core_ids=list(range(NCORES)))
    kernel._last_wall_s = _time.time() - _t0
    out = np.concatenate([r["out"].T for r in res.results], axis=0)  # (512, 10)
    kernel._last_res = res
    return out.astype(np.float32)


# revision 10
# speedup vs baseline: 1.0758x; 1.0758x over previous
"""Trainium2 Bass kernel for the spiking CNN (nn_CNNModel_47785806135777).

Key facts exploited:
  - The reference's straight-through graph is numerically identity in the
    forward pass: output == (Tf0 @ wf1.T) / (VTH * STEPS), where Tf0 is the
    fc-layer spike count.  Only the membranes + Tf0 must be computed.
  - The Poisson randomness is jax.random.key(42) threefry — bit-exactly
    reproducible on host CPU; spikes are precomputed and streamed in.
  - All matmul moving operands are binary spikes (exact in bf16); fp32
    weights are split into an exact 3-term bf16 sum, so every matmul runs
    at full bf16 PE speed with fp32-class accuracy.
  - avgpool is a linear map -> done on the tensor engine as a 0.25-valued
    matmul (exact in fp32 PSUM since spikes are 0/1).
Data parallel across 8 cores: 64 images per core.
"""

import numpy as np
import ml_dtypes

STEPS = 100
VTH = 1.0
POOL_TH = 0.75
B = 512
NCORES = 8
BL = B // NCORES  # 64

BF16 = ml_dtypes.bfloat16

_BUILD_CACHE = {}


# ---------------------------------------------------------------- host prep

def _spikes_all(inputdata):
    import jax
    import jax.numpy as jnp

    cpu = jax.devices("cpu")[0]
    with jax.default_device(cpu):
        keys = jax.random.split(jax.random.key(42), STEPS)
        x = jnp.asarray(inputdata)
        half = jnp.abs(x) / 2.0
        sgn = jnp.sign(x)

        @jax.jit
        def gen(keys):
            def body(_, k):
                r = jax.random.uniform(k, x.shape, dtype=jnp.float32)
                return None, (half > r).astype(jnp.float32) * sgn
            _, s = jax.lax.scan(body, None, keys)
            return s

        return np.asarray(gen(keys))  # (STEPS, B, 1, 28, 28)


def _split3(w):
    terms = []
    r = np.asarray(w, dtype=np.float32).copy()
    for _ in range(3):
        t = r.astype(BF16)
        terms.append(t)
        r = r - t.astype(np.float32)
    return terms


def _conv1_lhsT(w1s, ypack):
    # [50, 20*ypack]; k = dyp*5 + dx ; m = co*ypack + yp
    L = np.zeros((50, 20 * ypack), dtype=BF16)
    w = w1s  # bf16 (20,1,5,5)
    for dyp in range(10):
        for dx in range(5):
            k = dyp * 5 + dx
            for yp in range(ypack):
                dy = dyp - yp
                if 0 <= dy < 5:
                    for co in range(20):
                        L[k, co * ypack + yp] = w[co, 0, dy, dx]
    return L


def _conv2_lhsT(w2s, dx):
    # [120, 100]; k = dyp*20 + ci ; m = co*2 + yp
    L = np.zeros((120, 100), dtype=BF16)
    for dyp in range(6):
        for ci in range(20):
            k = dyp * 20 + ci
            for yp in range(2):
                dy = dyp - yp
                if 0 <= dy < 5:
                    for co in range(50):
                        L[k, co * 2 + yp] = w2s[co, ci, dy, dx]
    return L


def _pool_lhsT(nch, ypack):
    # [nch*ypack, nch*(ypack//2)] : 0.25 where co==co' and yp//2==y'p
    yo = ypack // 2
    L = np.zeros((nch * ypack, nch * yo), dtype=BF16)
    for co in range(nch):
        for yp in range(ypack):
            L[co * ypack + yp, co * yo + yp // 2] = 0.25
    return L


def _host_tensors(w1, w2, wf0, wf1):
    w1s = _split3(w1)
    w2s = _split3(w2)
    wf0s = _split3(wf0)

    # conv1: [3*50, 120] and [3*50, 80]
    c1w = np.concatenate([_conv1_lhsT(s, 6) for s in w1s], axis=0)
    c1wl = np.concatenate([_conv1_lhsT(s, 4) for s in w1s], axis=0)
    # conv2: [15*120, 100]  (s-major, then dx)
    c2w = np.concatenate(
        [_conv2_lhsT(w2s[s], dx) for s in range(3) for dx in range(5)], axis=0
    )
    p1w = _pool_lhsT(20, 6)      # [120, 60]
    p1wl = _pool_lhsT(20, 4)     # [80, 40]
    p2w = _pool_lhsT(50, 2)      # [100, 50]
    # fc: rows f = co*49 + y''*7 + x'' ; [3, 20, 128, 200] flattened
    fcw = np.zeros((3, 2560, 200), dtype=BF16)
    for s in range(3):
        fcw[s, :2450, :] = wf0s[s].T
    fcw = fcw.reshape(3 * 20, 128, 200).reshape(3 * 20 * 128, 200)
    # wf1 packed: [128, 20] fp32
    wf1p = np.zeros((128, 20), dtype=np.float32)
    wf1T = np.asarray(wf1, np.float32).T  # [200, 10]
    wf1p[:, 0:10] = wf1T[0:128]
    wf1p[:72, 10:20] = wf1T[128:200]
    return dict(c1w=c1w, c1wl=c1wl, c2w=c2w, p1w=p1w, p1wl=p1wl, p2w=p2w,
                fcw=fcw, wf1p=wf1p)


# ---------------------------------------------------------------- bass build

def _build(steps, stagger=False):
    import concourse.bass as bass
    import concourse.mybir as mybir
    import concourse.tile as tile
    from concourse import bacc

    dt = mybir.dt
    Alu = mybir.AluOpType

    nc = bacc.Bacc(trn_type="TRN2")

    spikes_h = nc.dram_tensor("spikes", [steps, 69632], dt.bfloat16, kind="ExternalInput")
    c1w_h = nc.dram_tensor("c1w", [150, 120], dt.bfloat16, kind="ExternalInput")
    c1wl_h = nc.dram_tensor("c1wl", [150, 80], dt.bfloat16, kind="ExternalInput")
    c2w_h = nc.dram_tensor("c2w", [1800, 100], dt.bfloat16, kind="ExternalInput")
    p1w_h = nc.dram_tensor("p1w", [120, 60], dt.bfloat16, kind="ExternalInput")
    p1wl_h = nc.dram_tensor("p1wl", [80, 40], dt.bfloat16, kind="ExternalInput")
    p2w_h = nc.dram_tensor("p2w", [100, 50], dt.bfloat16, kind="ExternalInput")
    fcw_h = nc.dram_tensor("fcw", [7680, 200], dt.bfloat16, kind="ExternalInput")
    wf1p_h = nc.dram_tensor("wf1p", [128, 20], dt.float32, kind="ExternalInput")
    out_h = nc.dram_tensor("out", [10, 64], dt.float32, kind="ExternalOutput")

    YB = [(0, 6), (1, 6), (2, 6), (3, 6), (4, 4)]  # (ybl, ypack)

    with tile.TileContext(nc) as tc:
        import contextlib
        ctx = contextlib.ExitStack()
        with ctx:
            state = ctx.enter_context(tc.tile_pool(name="state", bufs=1))
            dram = ctx.enter_context(tc.tile_pool(name="dram", bufs=1, space="DRAM"))
            work = ctx.enter_context(tc.tile_pool(name="work", bufs=2))
            patches_p = ctx.enter_context(tc.tile_pool(name="patches", bufs=3))
            o1_p = ctx.enter_context(tc.tile_pool(name="o1p", bufs=2))
            o3_p = ctx.enter_context(tc.tile_pool(name="o3p", bufs=2))
            ps_c1 = ctx.enter_context(tc.tile_pool(name="psc1", bufs=2, space="PSUM"))
            ps_c2 = ctx.enter_context(tc.tile_pool(name="psc2", bufs=2, space="PSUM"))
            ps_p1 = ctx.enter_context(tc.tile_pool(name="psp1", bufs=1, space="PSUM"))
            ps_p2 = ctx.enter_context(tc.tile_pool(name="psp2", bufs=1, space="PSUM"))
            ps_fc = ctx.enter_context(tc.tile_pool(name="psfc", bufs=1, space="PSUM"))
            ps_fc2 = ctx.enter_context(tc.tile_pool(name="psfc2", bufs=1, space="PSUM"))

            # ---- persistent state
            m1 = [state.tile([20 * yp, 1792], dt.float32, tag=f"m1_{i}", name=f"m1_{i}") for i, yp in YB]
            m1s = [state.tile([10 * yp, 896], dt.float32, tag=f"m1s_{i}", name=f"m1s_{i}") for i, yp in YB]
            m2 = state.tile([100, 6272], dt.float32, tag="m2", name="m2")
            m2s = state.tile([50, 3136], dt.float32, tag="m2s", name="m2s")
            mf0a = state.tile([128, 64], dt.float32, tag="mf0a", name="mf0a")
            mf0b = state.tile([72, 64], dt.float32, tag="mf0b", name="mf0b")
            tf0a = state.tile([128, 64], dt.float32, tag="tf0a", name="tf0a")
            tf0b = state.tile([72, 64], dt.float32, tag="tf0b", name="tf0b")
            zcol = state.tile([128, 1], dt.float32, tag="zcol", name="zcol")

            # ---- weights in SBUF
            c1w_sb = state.tile([50, 360], dt.bfloat16, tag="c1w", name="c1w")
            c1wl_sb = state.tile([50, 240], dt.bfloat16, tag="c1wl", name="c1wl")
            c2w_sb = state.tile([120, 1500], dt.bfloat16, tag="c2w", name="c2w")
            p1w_sb = state.tile([120, 60], dt.bfloat16, tag="p1w", name="p1w")
            p1wl_sb = state.tile([80, 40], dt.bfloat16, tag="p1wl", name="p1wl")
            p2w_sb = state.tile([100, 50], dt.bfloat16, tag="p2w", name="p2w")
            fcw_sb = state.tile([128, 12000], dt.bfloat16, tag="fcw", name="fcw")
            wf1p_sb = state.tile([128, 20], dt.float32, tag="wf1p", name="wf1p")

            # weight DMAs (reshape DRAM rows into sbuf free dims)
            for s in range(3):
                nc.sync.dma_start(out=c1w_sb[:, s * 120:(s + 1) * 120],
                                  in_=c1w_h[s * 50:(s + 1) * 50, :])
                nc.sync.dma_start(out=c1wl_sb[:, s * 80:(s + 1) * 80],
                                  in_=c1wl_h[s * 50:(s + 1) * 50, :])
            for j in range(15):
                nc.sync.dma_start(out=c2w_sb[:, j * 100:(j + 1) * 100],
                                  in_=c2w_h[j * 120:(j + 1) * 120, :])
            nc.sync.dma_start(out=p1w_sb[:], in_=p1w_h[:])
            nc.sync.dma_start(out=p1wl_sb[:], in_=p1wl_h[:])
            nc.sync.dma_start(out=p2w_sb[:], in_=p2w_h[:])
            for j in range(60):
                nc.sync.dma_start(out=fcw_sb[:, j * 200:(j + 1) * 200],
                                  in_=fcw_h[j * 128:(j + 1) * 128, :])
            nc.sync.dma_start(out=wf1p_sb[:], in_=wf1p_h[:])

            # ---- DRAM scratch
            o2_dram = dram.tile([20 * 18 * 18 * 64], dt.bfloat16, tag="o2d", name="o2d")
            o4_dram = dram.tile([2560 * 64], dt.bfloat16, tag="o4d", name="o4d")

            # ---- zero init
            for t in m1 + m1s + [m2, m2s, mf0a, mf0b, tf0a, tf0b, zcol]:
                nc.vector.memset(t[:], 0.0)
            zb = work.tile([128, 3240], dt.bfloat16, tag="zb", name="zb")
            nc.gpsimd.memset(zb[:], 0.0)
            nc.sync.dma_start(
                out=o2_dram.rearrange("(p f) -> p f", p=128), in_=zb[:])
            nc.sync.dma_start(
                out=o4_dram.rearrange("(p f) -> p f", p=128), in_=zb[:, :1280])

            def step_body(iv):
                # ---------------- conv1 + LIF1 + pool1 + LIFp1
                o2s_list = []
                for ybl, ypk in YB:
                    M = 20 * ypk
                    patches = patches_p.tile([50, 1792], dt.bfloat16, tag="patches", name="patches")
                    src = bass.AP(
                        tensor=spikes_h,
                        offset=iv * 69632 + 6 * ybl * 2048,
                        ap=[[2048, 10], [64, 5], [64, 28], [1, 64]],
                    )
                    nc.scalar.dma_start(out=patches[:], in_=src)
                    lw = c1w_sb if ypk == 6 else c1wl_sb
                    o1 = o1_p.tile([M, 1792], dt.bfloat16, tag="o1", name="o1")
                    for cp in range(2):
                        pss = [ps_c1.tile([M, 448], dt.float32, tag="psc1", name="psc1")
                               for _ in range(2)]
                        for s in range(3):
                            for ci in range(2):
                                c = cp * 2 + ci
                                nc.tensor.matmul(
                                    out=pss[ci][:],
                                    lhsT=lw[:, s * M:(s + 1) * M],
                                    rhs=patches[:, c * 448:(c + 1) * 448],
                                    start=(s == 0), stop=(s == 2),
                                )
                        for ci in range(2):
                            c = cp * 2 + ci
                            mm = m1[ybl][:, c * 448:(c + 1) * 448]
                            nc.vector.tensor_tensor(out=mm, in0=mm, in1=pss[ci][:], op=Alu.add)
                    # fire layer1
                    nc.gpsimd.tensor_scalar(out=o1[:], in0=m1[ybl][:], scalar1=VTH,
                                            scalar2=None, op0=Alu.is_gt)
                    nc.vector.copy_predicated(
                        out=m1[ybl][:], mask=o1[:].bitcast(dt.uint16),
                        data=zcol[0:M, 0:1].to_broadcast([M, 1792]))
                    # pool1 (PE): out [10*ypk, 448] x 2 chunks
                    pw = p1w_sb if ypk == 6 else p1wl_sb
                    MP = 10 * ypk
                    o1r = o1[:].rearrange("p (x two b) -> p x two b", two=2, b=64)
                    o2s = work.tile([MP, 896], dt.bfloat16, tag="o2s", name="o2s")
                    for c in range(2):
                        psp = ps_p1.tile([MP, 448], dt.float32, tag="psp1", name="psp1")
                        for par in range(2):
                            nc.tensor.matmul(
                                out=psp[:],
                                lhsT=pw[:],
                                rhs=o1r[:, c * 7:(c + 1) * 7, par, :],
                                start=(par == 0), stop=(par == 1),
                            )
                        mm = m1s[ybl][:, c * 448:(c + 1) * 448]
                        nc.vector.tensor_tensor(out=mm, in0=mm, in1=psp[:], op=Alu.add)
                    nc.gpsimd.tensor_scalar(out=o2s[:], in0=m1s[ybl][:], scalar1=POOL_TH,
                                            scalar2=None, op0=Alu.is_gt)
                    nc.vector.copy_predicated(
                        out=m1s[ybl][:], mask=o2s[:].bitcast(dt.uint16),
                        data=zcol[0:MP, 0:1].to_broadcast([MP, 896]))
                    o2s_list.append((ybl, ypk, o2s))

                # ---------------- o2 -> DRAM (padded [20, 18, 18, 64])
                for ybl, ypk, o2s in o2s_list:
                    yo = ypk // 2
                    dst = bass.AP(
                        tensor=o2_dram.tensor,
                        offset=o2_dram[:].offset + (3 * ybl + 2) * 1152 + 2 * 64,
                        ap=[[20736, 20], [1152, yo], [64, 14], [1, 64]],
                    )
                    srcr = o2s[:].rearrange("(co yo) (x b) -> (co yo) x b", yo=yo, b=64)
                    nc.gpsimd.dma_start(out=dst, in_=srcr)

                # ---------------- conv2 replica + conv2 + LIF2 + pool2 + LIFp2
                rep = work.tile([120, 8064], dt.bfloat16, tag="rep", name="rep")
                for dyp in range(6):
                    src = bass.AP(
                        tensor=o2_dram.tensor,
                        offset=o2_dram[:].offset + dyp * 1152,
                        ap=[[20736, 20], [2304, 7], [64, 18], [1, 64]],
                    )
                    eng = nc.scalar if dyp % 2 == 0 else nc.sync
                    eng.dma_start(out=rep[dyp * 20:(dyp + 1) * 20, :], in_=src)
                repr_ = rep[:].rearrange("p (y x b) -> p y x b", y=7, x=18, b=64)
                o4 = work.tile([50, 3136], dt.bfloat16, tag="o4", name="o4")
                for yb2 in range(7):
                    o3 = o3_p.tile([100, 896], dt.bfloat16, tag="o3", name="o3")
                    pss = [ps_c2.tile([100, 448], dt.float32, tag="psc2", name="psc2")
                           for _ in range(2)]
                    for s in range(3):
                        for dx in range(5):
                            for c in range(2):
                                nc.tensor.matmul(
                                    out=pss[c][:],
                                    lhsT=c2w_sb[:, (s * 5 + dx) * 100:(s * 5 + dx + 1) * 100],
                                    rhs=repr_[:, yb2, dx + c * 7:dx + (c + 1) * 7, :],
                                    start=(s == 0 and dx == 0), stop=(s == 2 and dx == 4),
                                )
                    for c in range(2):
                        mm = m2[:, yb2 * 896 + c * 448: yb2 * 896 + (c + 1) * 448]
                        nc.vector.tensor_tensor(out=mm, in0=mm, in1=pss[c][:], op=Alu.add)
                    m2c = m2[:, yb2 * 896:(yb2 + 1) * 896]
                    nc.gpsimd.tensor_scalar(out=o3[:], in0=m2c, scalar1=VTH,
                                            scalar2=None, op0=Alu.is_gt)
                    nc.vector.copy_predicated(
                        out=m2c, mask=o3[:].bitcast(dt.uint16),
                        data=zcol[0:100, 0:1].to_broadcast([100, 896]))
                    # pool2
                    o3r = o3[:].rearrange("p (x two b) -> p x two b", two=2, b=64)
                    psp = ps_p2.tile([50, 448], dt.float32, tag="psp2", name="psp2")
                    for par in range(2):
                        nc.tensor.matmul(out=psp[:], lhsT=p2w_sb[:],
                                         rhs=o3r[:, :, par, :],
                                         start=(par == 0), stop=(par == 1))
                    mm = m2s[:, yb2 * 448:(yb2 + 1) * 448]
                    nc.vector.tensor_tensor(out=mm, in0=mm, in1=psp[:], op=Alu.add)
                nc.gpsimd.tensor_scalar(out=o4[:], in0=m2s[:], scalar1=POOL_TH,
                                        scalar2=None, op0=Alu.is_gt)
                nc.vector.copy_predicated(
                    out=m2s[:], mask=o4[:].bitcast(dt.uint16),
                    data=zcol[0:50, 0:1].to_broadcast([50, 3136]))

                # ---------------- o4 -> DRAM -> o4T
                dst = bass.AP(tensor=o4_dram.tensor, offset=o4_dram[:].offset,
                              ap=[[3136, 50], [1, 3136]])
                nc.sync.dma_start(out=dst, in_=o4[:])
                o4T = work.tile([128, 1280], dt.bfloat16, tag="o4T", name="o4T")
                src = bass.AP(tensor=o4_dram.tensor, offset=o4_dram[:].offset,
                              ap=[[64, 128], [8192, 20], [1, 64]])
                nc.scalar.dma_start(out=o4T[:], in_=src)

                # ---------------- fc + LIF f0 + Tf0
                psa = ps_fc.tile([128, 64], dt.float32, tag="psfc", name="psfc")
                psb = ps_fc2.tile([72, 64], dt.float32, tag="psfc2", name="psfc2")
                for s in range(3):
                    for k in range(20):
                        base = (s * 20 + k) * 200
                        nc.tensor.matmul(out=psa[:],
                                         lhsT=fcw_sb[:, base:base + 128],
                                         rhs=o4T[:, k * 64:(k + 1) * 64],
                                         start=(s == 0 and k == 0),
                                         stop=(s == 2 and k == 19))
                        nc.tensor.matmul(out=psb[:],
                                         lhsT=fcw_sb[:, base + 128:base + 200],
                                         rhs=o4T[:, k * 64:(k + 1) * 64],
                                         start=(s == 0 and k == 0),
                                         stop=(s == 2 and k == 19))
                for mf, psx, tf, P in ((mf0a, psa, tf0a, 128), (mf0b, psb, tf0b, 72)):
                    nc.vector.tensor_tensor(out=mf[:], in0=mf[:], in1=psx[:], op=Alu.add)
                    o5 = work.tile([P, 64], dt.bfloat16, tag=f"o5_{P}", name=f"o5_{P}")
                    nc.vector.tensor_scalar(out=o5[:], in0=mf[:], scalar1=VTH,
                                            scalar2=None, op0=Alu.is_gt)
                    nc.vector.copy_predicated(
                        out=mf[:], mask=o5[:].bitcast(dt.uint16),
                        data=zcol[0:P, 0:1].to_broadcast([P, 64]))
                    nc.vector.tensor_tensor(out=tf[:], in0=tf[:], in1=o5[:], op=Alu.add)

            loop_kw = {}
            if stagger:
                loop_kw = dict(staggered_reset=True,
                               hint_engines=(mybir.EngineType.PE,
                                             mybir.EngineType.DVE,
                                             mybir.EngineType.Activation,
                                             mybir.EngineType.Pool,
                                             mybir.EngineType.SP))
            assert steps % 2 == 0
            with tc.For_i(0, steps, 2, **loop_kw) as iv:
                step_body(iv)
                step_body(iv + 1)

            # ---------------- final: out = (Tf0 @ wf1.T) / 100  -> [10, 64]
            pso = ps_fc.tile([10, 64], dt.float32, tag="psfc", name="psfc")
            nc.tensor.matmul(out=pso[:], lhsT=wf1p_sb[:, 0:10], rhs=tf0a[:],
                             start=True, stop=False)
            nc.tensor.matmul(out=pso[:], lhsT=wf1p_sb[0:72, 10:20], rhs=tf0b[:],
                             start=False, stop=True)
            out_sb = work.tile([10, 64], dt.float32, tag="outsb", name="outsb")
            nc.vector.tensor_scalar(out=out_sb[:], in0=pso[:],
                                    scalar1=1.0 / (VTH * STEPS), scalar2=None,
                                    op0=Alu.mult)
            nc.sync.dma_start(out=out_h[:], in_=out_sb[:])

    nc.finalize()
    return nc


# ---------------------------------------------------------------- entry

def kernel(inputdata, w1, w2, wf0, wf1, _steps=STEPS, _stagger=False):
    from concourse.bass_utils import run_bass_kernel_spmd

    spikes = _spikes_all(inputdata)  # (STEPS, 512, 1, 28, 28) fp32
    # pad into (steps, 32, 32, 64) per core, bf16
    hw = _host_tensors(w1, w2, wf0, wf1)

    key = (_steps, _stagger)
    if key not in _BUILD_CACHE:
        _BUILD_CACHE[key] = _build(_steps, _stagger)
    nc = _BUILD_CACHE[key]

    in_maps = []
    for c in range(NCORES):
        sp = np.zeros((_steps, 34, 32, BL), dtype=BF16)
        # spikes[t, b, 0, y, x] -> sp[t, y+2, x+2, b]
        blk = spikes[:_steps, c * BL:(c + 1) * BL, 0]  # (steps, 64, 28, 28)
        sp[:, 2:30, 2:30, :] = np.transpose(blk, (0, 2, 3, 1)).astype(BF16)
        m = dict(hw)
        m["spikes"] = sp.reshape(_steps, 69632)
        in_maps.append(m)

    import time as _time
    _t0 = _time.time()
    res = run_bass_kernel_spmd(nc, in_maps, core_ids=list(range(NCORES)))
    kernel._last_wall_s = _time.time() - _t0
    out = np.concatenate([r["out"].T for r in res.results], axis=0)  # (512, 10)
    kernel._last_res = res
    return out.astype(np.float32)


# revision 12
# speedup vs baseline: 189.0082x; 175.6945x over previous
"""Trainium2 Bass kernel for the spiking CNN (nn_CNNModel_47785806135777).

Key facts exploited:
  - The reference's straight-through graph is numerically identity in the
    forward pass: output == (Tf0 @ wf1.T) / (VTH * STEPS), where Tf0 is the
    fc-layer spike count.  Only the membranes + Tf0 must be computed.
  - The Poisson randomness is jax.random.key(42) threefry — bit-exactly
    reproducible on host CPU; spikes are precomputed and streamed in.
  - All matmul moving operands are binary spikes (exact in bf16); fp32
    weights are split into an exact 3-term bf16 sum, so every matmul runs
    at full bf16 PE speed with fp32-class accuracy.
  - avgpool is a linear map -> done on the tensor engine as a 0.25-valued
    matmul (exact in fp32 PSUM since spikes are 0/1).
Data parallel across 8 cores: 64 images per core.
"""

import numpy as np
import ml_dtypes

STEPS = 100
VTH = 1.0
POOL_TH = 0.75
B = 512
NCORES = 8
BL = B // NCORES  # 64

BF16 = ml_dtypes.bfloat16

_BUILD_CACHE = {}


# ---------------------------------------------------------------- host prep

def _spikes_all(inputdata):
    import jax
    import jax.numpy as jnp

    cpu = jax.devices("cpu")[0]
    with jax.default_device(cpu):
        keys = jax.random.split(jax.random.key(42), STEPS)
        x = jnp.asarray(inputdata)
        half = jnp.abs(x) / 2.0
        sgn = jnp.sign(x)

        @jax.jit
        def gen(keys):
            def body(_, k):
                r = jax.random.uniform(k, x.shape, dtype=jnp.float32)
                return None, (half > r).astype(jnp.float32) * sgn
            _, s = jax.lax.scan(body, None, keys)
            return s

        return np.asarray(gen(keys))  # (STEPS, B, 1, 28, 28)


def _split3(w):
    terms = []
    r = np.asarray(w, dtype=np.float32).copy()
    for _ in range(3):
        t = r.astype(BF16)
        terms.append(t)
        r = r - t.astype(np.float32)
    return terms


def _conv1_lhsT(w1s, ypack):
    # [50, 20*ypack]; k = dyp*5 + dx ; m = co*ypack + yp
    L = np.zeros((50, 20 * ypack), dtype=BF16)
    w = w1s  # bf16 (20,1,5,5)
    for dyp in range(10):
        for dx in range(5):
            k = dyp * 5 + dx
            for yp in range(ypack):
                dy = dyp - yp
                if 0 <= dy < 5:
                    for co in range(20):
                        L[k, co * ypack + yp] = w[co, 0, dy, dx]
    return L


def _conv2_lhsT(w2s, dx):
    # [120, 100]; k = dyp*20 + ci ; m = co*2 + yp
    L = np.zeros((120, 100), dtype=BF16)
    for dyp in range(6):
        for ci in range(20):
            k = dyp * 20 + ci
            for yp in range(2):
                dy = dyp - yp
                if 0 <= dy < 5:
                    for co in range(50):
                        L[k, co * 2 + yp] = w2s[co, ci, dy, dx]
    return L


def _pool_lhsT(nch, ypack):
    # [nch*ypack, nch*(ypack//2)] : 0.25 where co==co' and yp//2==y'p
    yo = ypack // 2
    L = np.zeros((nch * ypack, nch * yo), dtype=BF16)
    for co in range(nch):
        for yp in range(ypack):
            L[co * ypack + yp, co * yo + yp // 2] = 0.25
    return L


def _host_tensors(w1, w2, wf0, wf1):
    w1s = _split3(w1)
    w2s = _split3(w2)
    wf0s = _split3(wf0)

    # conv1: [3*50, 120] and [3*50, 80]
    c1w = np.concatenate([_conv1_lhsT(s, 6) for s in w1s], axis=0)
    c1wl = np.concatenate([_conv1_lhsT(s, 4) for s in w1s], axis=0)
    # conv2: [15*120, 100]  (s-major, then dx)
    c2w = np.concatenate(
        [_conv2_lhsT(w2s[s], dx) for s in range(3) for dx in range(5)], axis=0
    )
    p1w = _pool_lhsT(20, 6)      # [120, 60]
    p1wl = _pool_lhsT(20, 4)     # [80, 40]
    p2w = _pool_lhsT(50, 2)      # [100, 50]
    # fc: rows f = co*49 + y''*7 + x'' ; [3, 20, 128, 200] flattened
    fcw = np.zeros((3, 2560, 200), dtype=BF16)
    for s in range(3):
        fcw[s, :2450, :] = wf0s[s].T
    fcw = fcw.reshape(3 * 20, 128, 200).reshape(3 * 20 * 128, 200)
    # wf1 packed: [128, 20] fp32
    wf1p = np.zeros((128, 20), dtype=np.float32)
    wf1T = np.asarray(wf1, np.float32).T  # [200, 10]
    wf1p[:, 0:10] = wf1T[0:128]
    wf1p[:72, 10:20] = wf1T[128:200]
    return dict(c1w=c1w, c1wl=c1wl, c2w=c2w, p1w=p1w, p1wl=p1wl, p2w=p2w,
                fcw=fcw, wf1p=wf1p)


# ---------------------------------------------------------------- bass build

def _build(steps, stagger=False, UNROLL=2):
    import concourse.bass as bass
    import concourse.mybir as mybir
    import concourse.tile as tile
    from concourse import bacc

    dt = mybir.dt
    Alu = mybir.AluOpType

    nc = bacc.Bacc(trn_type="TRN2")

    spikes_h = nc.dram_tensor("spikes", [steps, 69632], dt.bfloat16, kind="ExternalInput")
    c1w_h = nc.dram_tensor("c1w", [150, 120], dt.bfloat16, kind="ExternalInput")
    c1wl_h = nc.dram_tensor("c1wl", [150, 80], dt.bfloat16, kind="ExternalInput")
    c2w_h = nc.dram_tensor("c2w", [1800, 100], dt.bfloat16, kind="ExternalInput")
    p1w_h = nc.dram_tensor("p1w", [120, 60], dt.bfloat16, kind="ExternalInput")
    p1wl_h = nc.dram_tensor("p1wl", [80, 40], dt.bfloat16, kind="ExternalInput")
    p2w_h = nc.dram_tensor("p2w", [100, 50], dt.bfloat16, kind="ExternalInput")
    fcw_h = nc.dram_tensor("fcw", [7680, 200], dt.bfloat16, kind="ExternalInput")
    wf1p_h = nc.dram_tensor("wf1p", [128, 20], dt.float32, kind="ExternalInput")
    out_h = nc.dram_tensor("out", [10, 64], dt.float32, kind="ExternalOutput")

    YB = [(0, 6), (1, 6), (2, 6), (3, 6), (4, 4)]  # (ybl, ypack)

    with tile.TileContext(nc) as tc:
        import contextlib
        ctx = contextlib.ExitStack()
        with ctx:
            state = ctx.enter_context(tc.tile_pool(name="state", bufs=1))
            dram = ctx.enter_context(tc.tile_pool(name="dram", bufs=1, space="DRAM"))
            work = ctx.enter_context(tc.tile_pool(name="work", bufs=2))
            patches_p = ctx.enter_context(tc.tile_pool(name="patches", bufs=3))
            o1_p = ctx.enter_context(tc.tile_pool(name="o1p", bufs=2))
            o3_p = ctx.enter_context(tc.tile_pool(name="o3p", bufs=2))
            ps_c1 = ctx.enter_context(tc.tile_pool(name="psc1", bufs=2, space="PSUM"))
            ps_c2 = ctx.enter_context(tc.tile_pool(name="psc2", bufs=2, space="PSUM"))
            ps_p1 = ctx.enter_context(tc.tile_pool(name="psp1", bufs=1, space="PSUM"))
            ps_p2 = ctx.enter_context(tc.tile_pool(name="psp2", bufs=1, space="PSUM"))
            ps_fc = ctx.enter_context(tc.tile_pool(name="psfc", bufs=1, space="PSUM"))
            ps_fc2 = ctx.enter_context(tc.tile_pool(name="psfc2", bufs=1, space="PSUM"))

            # ---- persistent state
            m1 = [state.tile([20 * yp, 1792], dt.float32, tag=f"m1_{i}", name=f"m1_{i}") for i, yp in YB]
            m1s = [state.tile([10 * yp, 896], dt.float32, tag=f"m1s_{i}", name=f"m1s_{i}") for i, yp in YB]
            m2 = state.tile([100, 6272], dt.float32, tag="m2", name="m2")
            m2s = state.tile([50, 3136], dt.float32, tag="m2s", name="m2s")
            mf0a = state.tile([128, 64], dt.float32, tag="mf0a", name="mf0a")
            mf0b = state.tile([72, 64], dt.float32, tag="mf0b", name="mf0b")
            tf0a = state.tile([128, 64], dt.float32, tag="tf0a", name="tf0a")
            tf0b = state.tile([72, 64], dt.float32, tag="tf0b", name="tf0b")
            zcol = state.tile([128, 1], dt.float32, tag="zcol", name="zcol")

            # ---- weights in SBUF
            c1w_sb = state.tile([50, 360], dt.bfloat16, tag="c1w", name="c1w")
            c1wl_sb = state.tile([50, 240], dt.bfloat16, tag="c1wl", name="c1wl")
            c2w_sb = state.tile([120, 1500], dt.bfloat16, tag="c2w", name="c2w")
            p1w_sb = state.tile([120, 60], dt.bfloat16, tag="p1w", name="p1w")
            p1wl_sb = state.tile([80, 40], dt.bfloat16, tag="p1wl", name="p1wl")
            p2w_sb = state.tile([100, 50], dt.bfloat16, tag="p2w", name="p2w")
            fcw_sb = state.tile([128, 12000], dt.bfloat16, tag="fcw", name="fcw")
            wf1p_sb = state.tile([128, 20], dt.float32, tag="wf1p", name="wf1p")

            # weight DMAs (reshape DRAM rows into sbuf free dims)
            for s in range(3):
                nc.sync.dma_start(out=c1w_sb[:, s * 120:(s + 1) * 120],
                                  in_=c1w_h[s * 50:(s + 1) * 50, :])
                nc.sync.dma_start(out=c1wl_sb[:, s * 80:(s + 1) * 80],
                                  in_=c1wl_h[s * 50:(s + 1) * 50, :])
            for j in range(15):
                nc.sync.dma_start(out=c2w_sb[:, j * 100:(j + 1) * 100],
                                  in_=c2w_h[j * 120:(j + 1) * 120, :])
            nc.sync.dma_start(out=p1w_sb[:], in_=p1w_h[:])
            nc.sync.dma_start(out=p1wl_sb[:], in_=p1wl_h[:])
            nc.sync.dma_start(out=p2w_sb[:], in_=p2w_h[:])
            for j in range(60):
                nc.sync.dma_start(out=fcw_sb[:, j * 200:(j + 1) * 200],
                                  in_=fcw_h[j * 128:(j + 1) * 128, :])
            nc.sync.dma_start(out=wf1p_sb[:], in_=wf1p_h[:])

            # ---- DRAM scratch
            o2_dram = dram.tile([20 * 18 * 18 * 64], dt.bfloat16, tag="o2d", name="o2d")
            o4_dram = dram.tile([2560 * 64], dt.bfloat16, tag="o4d", name="o4d")

            # ---- zero init
            for t in m1 + m1s + [m2, m2s, mf0a, mf0b, tf0a, tf0b, zcol]:
                nc.vector.memset(t[:], 0.0)
            zb = work.tile([128, 3240], dt.bfloat16, tag="zb", name="zb")
            nc.gpsimd.memset(zb[:], 0.0)
            nc.sync.dma_start(
                out=o2_dram.rearrange("(p f) -> p f", p=128), in_=zb[:])
            nc.sync.dma_start(
                out=o4_dram.rearrange("(p f) -> p f", p=128), in_=zb[:, :1280])

            def step_body(iv):
                # ---------------- conv1 + LIF1 + pool1 + LIFp1
                o2s_list = []
                for ybl, ypk in YB:
                    M = 20 * ypk
                    patches = patches_p.tile([50, 1792], dt.bfloat16, tag="patches", name="patches")
                    src = bass.AP(
                        tensor=spikes_h,
                        offset=iv * 69632 + 6 * ybl * 2048,
                        ap=[[2048, 10], [64, 5], [64, 28], [1, 64]],
                    )
                    nc.scalar.dma_start(out=patches[:], in_=src)
                    lw = c1w_sb if ypk == 6 else c1wl_sb
                    o1 = o1_p.tile([M, 1792], dt.bfloat16, tag="o1", name="o1")
                    for cp in range(2):
                        pss = [ps_c1.tile([M, 448], dt.float32, tag="psc1", name="psc1")
                               for _ in range(2)]
                        for s in range(3):
                            for ci in range(2):
                                c = cp * 2 + ci
                                nc.tensor.matmul(
                                    out=pss[ci][:],
                                    lhsT=lw[:, s * M:(s + 1) * M],
                                    rhs=patches[:, c * 448:(c + 1) * 448],
                                    start=(s == 0), stop=(s == 2),
                                )
                        for ci in range(2):
                            c = cp * 2 + ci
                            mm = m1[ybl][:, c * 448:(c + 1) * 448]
                            nc.vector.tensor_tensor(out=mm, in0=mm, in1=pss[ci][:], op=Alu.add)
                    # fire layer1
                    nc.gpsimd.tensor_scalar(out=o1[:], in0=m1[ybl][:], scalar1=VTH,
                                            scalar2=None, op0=Alu.is_gt)
                    nc.vector.copy_predicated(
                        out=m1[ybl][:], mask=o1[:].bitcast(dt.uint16),
                        data=zcol[0:M, 0:1].to_broadcast([M, 1792]))
                    # pool1 (PE): out [10*ypk, 448] x 2 chunks
                    pw = p1w_sb if ypk == 6 else p1wl_sb
                    MP = 10 * ypk
                    o1r = o1[:].rearrange("p (x two b) -> p x two b", two=2, b=64)
                    o2s = work.tile([MP, 896], dt.bfloat16, tag="o2s", name="o2s")
                    for c in range(2):
                        psp = ps_p1.tile([MP, 448], dt.float32, tag="psp1", name="psp1")
                        for par in range(2):
                            nc.tensor.matmul(
                                out=psp[:],
                                lhsT=pw[:],
                                rhs=o1r[:, c * 7:(c + 1) * 7, par, :],
                                start=(par == 0), stop=(par == 1),
                            )
                        mm = m1s[ybl][:, c * 448:(c + 1) * 448]
                        nc.vector.tensor_tensor(out=mm, in0=mm, in1=psp[:], op=Alu.add)
                    nc.gpsimd.tensor_scalar(out=o2s[:], in0=m1s[ybl][:], scalar1=POOL_TH,
                                            scalar2=None, op0=Alu.is_gt)
                    nc.vector.copy_predicated(
                        out=m1s[ybl][:], mask=o2s[:].bitcast(dt.uint16),
                        data=zcol[0:MP, 0:1].to_broadcast([MP, 896]))
                    o2s_list.append((ybl, ypk, o2s))

                # ---------------- o2 -> DRAM (padded [20, 18, 18, 64])
                for ybl, ypk, o2s in o2s_list:
                    yo = ypk // 2
                    dst = bass.AP(
                        tensor=o2_dram.tensor,
                        offset=o2_dram[:].offset + (3 * ybl + 2) * 1152 + 2 * 64,
                        ap=[[20736, 20], [1152, yo], [64, 14], [1, 64]],
                    )
                    srcr = o2s[:].rearrange("(co yo) (x b) -> (co yo) x b", yo=yo, b=64)
                    nc.gpsimd.dma_start(out=dst, in_=srcr)

                # ---------------- conv2 replica + conv2 + LIF2 + pool2 + LIFp2
                rep = work.tile([120, 8064], dt.bfloat16, tag="rep", name="rep")
                for dyp in range(6):
                    src = bass.AP(
                        tensor=o2_dram.tensor,
                        offset=o2_dram[:].offset + dyp * 1152,
                        ap=[[20736, 20], [2304, 7], [64, 18], [1, 64]],
                    )
                    eng = nc.scalar if dyp % 2 == 0 else nc.sync
                    eng.dma_start(out=rep[dyp * 20:(dyp + 1) * 20, :], in_=src)
                repr_ = rep[:].rearrange("p (y x b) -> p y x b", y=7, x=18, b=64)
                o4 = work.tile([50, 3136], dt.bfloat16, tag="o4", name="o4")
                for yb2 in range(7):
                    o3 = o3_p.tile([100, 896], dt.bfloat16, tag="o3", name="o3")
                    pss = [ps_c2.tile([100, 448], dt.float32, tag="psc2", name="psc2")
                           for _ in range(2)]
                    for s in range(3):
                        for dx in range(5):
                            for c in range(2):
                                nc.tensor.matmul(
                                    out=pss[c][:],
                                    lhsT=c2w_sb[:, (s * 5 + dx) * 100:(s * 5 + dx + 1) * 100],
                                    rhs=repr_[:, yb2, dx + c * 7:dx + (c + 1) * 7, :],
                                    start=(s == 0 and dx == 0), stop=(s == 2 and dx == 4),
                                )
                    for c in range(2):
                        mm = m2[:, yb2 * 896 + c * 448: yb2 * 896 + (c + 1) * 448]
                        nc.vector.tensor_tensor(out=mm, in0=mm, in1=pss[c][:], op=Alu.add)
                    m2c = m2[:, yb2 * 896:(yb2 + 1) * 896]
                    nc.gpsimd.tensor_scalar(out=o3[:], in0=m2c, scalar1=VTH,
                                            scalar2=None, op0=Alu.is_gt)
                    nc.vector.copy_predicated(
                        out=m2c, mask=o3[:].bitcast(dt.uint16),
                        data=zcol[0:100, 0:1].to_broadcast([100, 896]))
                    # pool2
                    o3r = o3[:].rearrange("p (x two b) -> p x two b", two=2, b=64)
                    psp = ps_p2.tile([50, 448], dt.float32, tag="psp2", name="psp2")
                    for par in range(2):
                        nc.tensor.matmul(out=psp[:], lhsT=p2w_sb[:],
                                         rhs=o3r[:, :, par, :],
                                         start=(par == 0), stop=(par == 1))
                    mm = m2s[:, yb2 * 448:(yb2 + 1) * 448]
                    nc.vector.tensor_tensor(out=mm, in0=mm, in1=psp[:], op=Alu.add)
                nc.gpsimd.tensor_scalar(out=o4[:], in0=m2s[:], scalar1=POOL_TH,
                                        scalar2=None, op0=Alu.is_gt)
                nc.vector.copy_predicated(
                    out=m2s[:], mask=o4[:].bitcast(dt.uint16),
                    data=zcol[0:50, 0:1].to_broadcast([50, 3136]))

                # ---------------- o4 -> DRAM -> o4T
                dst = bass.AP(tensor=o4_dram.tensor, offset=o4_dram[:].offset,
                              ap=[[3136, 50], [1, 3136]])
                nc.sync.dma_start(out=dst, in_=o4[:])
                o4T = work.tile([128, 1280], dt.bfloat16, tag="o4T", name="o4T")
                src = bass.AP(tensor=o4_dram.tensor, offset=o4_dram[:].offset,
                              ap=[[64, 128], [8192, 20], [1, 64]])
                nc.scalar.dma_start(out=o4T[:], in_=src)

                # ---------------- fc + LIF f0 + Tf0
                psa = ps_fc.tile([128, 64], dt.float32, tag="psfc", name="psfc")
                psb = ps_fc2.tile([72, 64], dt.float32, tag="psfc2", name="psfc2")
                for s in range(3):
                    for k in range(20):
                        base = (s * 20 + k) * 200
                        nc.tensor.matmul(out=psa[:],
                                         lhsT=fcw_sb[:, base:base + 128],
                                         rhs=o4T[:, k * 64:(k + 1) * 64],
                                         start=(s == 0 and k == 0),
                                         stop=(s == 2 and k == 19))
                        nc.tensor.matmul(out=psb[:],
                                         lhsT=fcw_sb[:, base + 128:base + 200],
                                         rhs=o4T[:, k * 64:(k + 1) * 64],
                                         start=(s == 0 and k == 0),
                                         stop=(s == 2 and k == 19))
                for mf, psx, tf, P in ((mf0a, psa, tf0a, 128), (mf0b, psb, tf0b, 72)):
                    nc.vector.tensor_tensor(out=mf[:], in0=mf[:], in1=psx[:], op=Alu.add)
                    o5 = work.tile([P, 64], dt.bfloat16, tag=f"o5_{P}", name=f"o5_{P}")
                    nc.vector.tensor_scalar(out=o5[:], in0=mf[:], scalar1=VTH,
                                            scalar2=None, op0=Alu.is_gt)
                    nc.vector.copy_predicated(
                        out=mf[:], mask=o5[:].bitcast(dt.uint16),
                        data=zcol[0:P, 0:1].to_broadcast([P, 64]))
                    nc.vector.tensor_tensor(out=tf[:], in0=tf[:], in1=o5[:], op=Alu.add)

            loop_kw = {}
            if stagger:
                loop_kw = dict(staggered_reset=True,
                               hint_engines=(mybir.EngineType.PE,
                                             mybir.EngineType.DVE,
                                             mybir.EngineType.Activation,
                                             mybir.EngineType.Pool,
                                             mybir.EngineType.SP))
            assert steps % UNROLL == 0
            with tc.For_i(0, steps, UNROLL, **loop_kw) as iv:
                for u in range(UNROLL):
                    step_body(iv + u if u else iv)

            # ---------------- final: out = (Tf0 @ wf1.T) / 100  -> [10, 64]
            pso = ps_fc.tile([10, 64], dt.float32, tag="psfc", name="psfc")
            nc.tensor.matmul(out=pso[:], lhsT=wf1p_sb[:, 0:10], rhs=tf0a[:],
                             start=True, stop=False)
            nc.tensor.matmul(out=pso[:], lhsT=wf1p_sb[0:72, 10:20], rhs=tf0b[:],
                             start=False, stop=True)
            out_sb = work.tile([10, 64], dt.float32, tag="outsb", name="outsb")
            nc.vector.tensor_scalar(out=out_sb[:], in0=pso[:],
                                    scalar1=1.0 / (VTH * STEPS), scalar2=None,
                                    op0=Alu.mult)
            nc.sync.dma_start(out=out_h[:], in_=out_sb[:])

    nc.finalize()
    return nc


# ---------------------------------------------------------------- entry

def kernel(inputdata, w1, w2, wf0, wf1, _steps=STEPS, _stagger=False, _unroll=4):
    from concourse.bass_utils import run_bass_kernel_spmd

    spikes = _spikes_all(inputdata)  # (STEPS, 512, 1, 28, 28) fp32
    # pad into (steps, 32, 32, 64) per core, bf16
    hw = _host_tensors(w1, w2, wf0, wf1)

    key = (_steps, _stagger, _unroll)
    if key not in _BUILD_CACHE:
        _BUILD_CACHE[key] = _build(_steps, _stagger, _unroll)
    nc = _BUILD_CACHE[key]

    in_maps = []
    for c in range(NCORES):
        sp = np.zeros((_steps, 34, 32, BL), dtype=BF16)
        # spikes[t, b, 0, y, x] -> sp[t, y+2, x+2, b]
        blk = spikes[:_steps, c * BL:(c + 1) * BL, 0]  # (steps, 64, 28, 28)
        sp[:, 2:30, 2:30, :] = np.transpose(blk, (0, 2, 3, 1)).astype(BF16)
        m = dict(hw)
        m["spikes"] = sp.reshape(_steps, 69632)
        in_maps.append(m)

    import time as _time
    _t0 = _time.time()
    res = run_bass_kernel_spmd(nc, in_maps, core_ids=list(range(NCORES)))
    kernel._last_wall_s = _time.time() - _t0
    out = np.concatenate([r["out"].T for r in res.results], axis=0)  # (512, 10)
    kernel._last_res = res
    return out.astype(np.float32)
